# revision 5
# baseline (speedup 1.0000x reference)
"""Trainium2 Bass kernel for a 2-layer GNN (dense message passing) + MLP readout.

Reference computation (N=16384 nodes, D=64 features, G=128 graphs):
    adj_hat = adj + I
    x  = adj_hat @ x_in                 # prop 1
    x  = relu(x @ W1.T + b1)            # fc1
    x  = adj_hat @ x                    # prop 2
    x  = relu(x @ W2.T + b2)            # fc2
    out = segment_sum(x, idx, 128)      # readout
    out = relu(out @ W3.T + b3)
    out = out @ W4.T + b4
    return log_softmax(out, axis=1)

Sharding over 8 NeuronCores: row-shard adj_hat over output nodes (2048 rows
per core). Each core receives its shard pre-transposed (A_T[j, i_local], with
self-loops folded in on the host) so the contraction index j lands on SBUF
partitions. The propagation matmul streams A_T tiles as the moving operand
with x chunks as the stationary operand, producing h.T in PSUM. The small fc
layers run with the bias folded in via an appended ones-row. h1 is
all-gathered between the propagation steps; the segment-sum readout uses a
host-built one-hot matrix as a matmul and an all-reduce of the [64, 128]
partial, after which every core computes the tiny readout MLP + log_softmax.
"""

import os
import sys

for _p in ("/opt/trn_rl_repo",):
    if _p not in sys.path and os.path.isdir(_p):
        sys.path.insert(0, _p)

import numpy as np

import concourse.bass as bass
import concourse.bacc as bacc
import concourse.tile as tile
import concourse.mybir as mybir
from concourse.bass_utils import run_bass_kernel_spmd

F32 = mybir.dt.float32

N = 16384          # nodes
D = 64             # feature dim (== H1 == H2 == H3)
G = 128            # graphs
NCLASS = 10
NCORES = 8
ROWS = N // NCORES          # 2048 output rows per core
JCHUNK = 128                # contraction tile (partition dim)
NJC = N // JCHUNK           # 128 j-chunks
NBANK = 4                   # psum accumulator banks per propagation
BCOLS = ROWS // NBANK       # 512 columns per bank
IB = ROWS // 128            # 16 output-row blocks of 128

# dtype of the adjacency streaming path (adj shard, x/h1 matmul weights).
# "float32r" is bit-identical to float32 in memory but streams through the
# PE at full rate (fp32 runs at 1/4 rate).
ADJ_DT_NAME = "float32r"

_BUILD_CACHE = {}


def _adj_dt():
    return getattr(mybir.dt, ADJ_DT_NAME)


def _np_adj_dt():
    return mybir.dt.np(_adj_dt())


def _build(adj_dt, reps=1):
    nc = bacc.Bacc("TRN2", target_bir_lowering=False, debug=False,
                   enable_asserts=True, num_devices=NCORES)

    at = nc.dram_tensor("at", [N, ROWS], adj_dt, kind="ExternalInput").ap()
    xw = nc.dram_tensor("xw", [128, (N // 128) * D], adj_dt, kind="ExternalInput").ap()
    st = nc.dram_tensor("st", [128, IB * G], F32, kind="ExternalInput").ap()
    w1t = nc.dram_tensor("w1t", [D + 1, D], F32, kind="ExternalInput").ap()
    w2t = nc.dram_tensor("w2t", [D + 1, D], F32, kind="ExternalInput").ap()
    w3t = nc.dram_tensor("w3t", [D, D], F32, kind="ExternalInput").ap()
    b3 = nc.dram_tensor("b3", [D, 1], F32, kind="ExternalInput").ap()
    w4t = nc.dram_tensor("w4t", [D, NCLASS], F32, kind="ExternalInput").ap()
    b4 = nc.dram_tensor("b4", [NCLASS, 1], F32, kind="ExternalInput").ap()
    eye = nc.dram_tensor("eye", [32, 32], F32, kind="ExternalInput").ap()
    out = nc.dram_tensor("out", [G, NCLASS], F32, kind="ExternalOutput").ap()

    groups = [list(range(NCORES))]

    with tile.TileContext(nc) as tc:
        with (
            tc.tile_pool(name="const", bufs=1) as const,
            tc.tile_pool(name="astream", bufs=8) as astream,
            tc.tile_pool(name="wstream", bufs=4) as wstream,
            tc.tile_pool(name="hp_pool", bufs=2) as hp_pool,
            tc.tile_pool(name="hb_pool", bufs=4) as hb_pool,
            tc.tile_pool(name="small", bufs=1) as small,
            tc.tile_pool(name="acc_pool", bufs=4, space="PSUM") as acc_pool,
            tc.tile_pool(name="pf_pool", bufs=2, space="PSUM") as pf_pool,
            tc.tile_pool(name="seg_pool", bufs=1, space="PSUM") as seg_pool,
            tc.tile_pool(name="dram", bufs=1, space="DRAM") as dram,
        ):
            # ---- constants into SBUF ----
            x_all = const.tile([128, (N // 128) * D], adj_dt)
            nc.sync.dma_start(x_all[:], xw[:])
            st_all = const.tile([128, IB * G], F32)
            nc.sync.dma_start(st_all[:], st[:])
            w1t_s = const.tile([D + 1, D], F32)
            nc.sync.dma_start(w1t_s[:], w1t[:])
            w2t_s = const.tile([D + 1, D], F32)
            nc.sync.dma_start(w2t_s[:], w2t[:])
            w3t_s = const.tile([D, D], F32)
            nc.sync.dma_start(w3t_s[:], w3t[:])
            b3_s = const.tile([D, 1], F32)
            nc.sync.dma_start(b3_s[:], b3[:])
            w4t_s = const.tile([D, NCLASS], F32)
            nc.sync.dma_start(w4t_s[:], w4t[:])
            b4_s = const.tile([NCLASS, 1], F32)
            nc.sync.dma_start(b4_s[:], b4[:])
            eye_s = const.tile([32, 32], F32)
            nc.sync.dma_start(eye_s[:], eye[:])

            def propagate(weight_of_chunk):
                """One propagation: h.T = sum_j w[j,:].T * A_T[j,:] into PSUM,
                then copied to a [65, 2048] SBUF tile with a trailing ones row
                (for the bias-folding fc matmul). Returns the SBUF tile."""
                acc = [
                    acc_pool.tile([D, BCOLS], F32, name=f"acc{b}", tag="acc")
                    for b in range(NBANK)
                ]
                for jc in range(NJC):
                    a_t = astream.tile([128, ROWS], adj_dt, name="a_t", tag="a")
                    nc.sync.dma_start(a_t[:], at[jc * 128:(jc + 1) * 128, :])
                    wsl = weight_of_chunk(jc)
                    for b in range(NBANK):
                        nc.tensor.matmul(
                            acc[b][:],
                            wsl,
                            a_t[:, b * BCOLS:(b + 1) * BCOLS],
                            start=(jc == 0),
                            stop=(jc == NJC - 1),
                        )
                hp = hp_pool.tile([D + 1, ROWS], F32, name="hp", tag="hp")
                nc.vector.memset(hp[D:D + 1, :], 1.0)
                for b in range(NBANK):
                    nc.vector.tensor_copy(hp[0:D, b * BCOLS:(b + 1) * BCOLS], acc[b][:])
                return hp

            for _rep in range(reps):
                h1_loc = dram.tile([ROWS, D], adj_dt, name="h1_loc")
                h1_full = dram.tile([N, D], adj_dt, name="h1_full",
                                    addr_space="Shared")
                seg_loc = dram.tile([D, G], F32, name="seg_loc")
                seg_full = dram.tile([D, G], F32, name="seg_full",
                                     addr_space="Shared")

                # ---- propagation 1 (weights: x chunks already resident) ----
                hp1 = propagate(lambda jc: x_all[:, jc * D:(jc + 1) * D])

                # ---- fc1 (+bias via ones row) -> relu -> h1 natural -> DRAM ----
                for ib in range(IB):
                    pf = pf_pool.tile([128, D], F32, name="pf1", tag="pf")
                    nc.tensor.matmul(pf[:], hp1[:, ib * 128:(ib + 1) * 128], w1t_s[:],
                                     start=True, stop=True)
                    hb = hb_pool.tile([128, D], adj_dt, name="hb1", tag="hb1")
                    nc.scalar.activation(hb[:], pf[:],
                                         mybir.ActivationFunctionType.Relu)
                    nc.sync.dma_start(h1_loc[ib * 128:(ib + 1) * 128, :], hb[:])

                nc.gpsimd.collective_compute(
                    "AllGather", mybir.AluOpType.bypass, replica_groups=groups,
                    ins=[h1_loc.opt()], outs=[h1_full.opt()],
                )

                # ---- propagation 2 (weights: gathered h1 chunks) ----
                def h1_weight(jc):
                    w_t = wstream.tile([128, D], adj_dt, name="w_t", tag="w")
                    nc.sync.dma_start(w_t[:], h1_full[jc * 128:(jc + 1) * 128, :])
                    return w_t[:]

                hp2 = propagate(h1_weight)

                # ---- fc2 -> relu -> h2 natural; readout partial via one-hot ----
                seg_ps = seg_pool.tile([D, G], F32, name="seg_ps", tag="seg")
                for ib in range(IB):
                    pf = pf_pool.tile([128, D], F32, name="pf2", tag="pf")
                    nc.tensor.matmul(pf[:], hp2[:, ib * 128:(ib + 1) * 128], w2t_s[:],
                                     start=True, stop=True)
                    hb2 = hb_pool.tile([128, D], F32, name="hb2", tag="hb2")
                    nc.scalar.activation(hb2[:], pf[:],
                                         mybir.ActivationFunctionType.Relu)
                    nc.tensor.matmul(seg_ps[:], hb2[:], st_all[:, ib * G:(ib + 1) * G],
                                     start=(ib == 0), stop=(ib == IB - 1))

                seg_s = small.tile([D, G], F32, name="seg_s", tag="seg_s")
                nc.vector.tensor_copy(seg_s[:], seg_ps[:])
                nc.sync.dma_start(seg_loc[:], seg_s[:])
                nc.gpsimd.collective_compute(
                    "AllReduce", mybir.AluOpType.add, replica_groups=groups,
                    ins=[seg_loc.opt()], outs=[seg_full.opt()],
                )
                segf_s = small.tile([D, G], F32, name="segf_s", tag="segf_s")
                nc.sync.dma_start(segf_s[:], seg_full[:])

                # ---- readout MLP: fc3 relu, fc4 (+bias), all in .T layout ----
                p3 = pf_pool.tile([D, G], F32, name="p3", tag="pf")
                nc.tensor.matmul(p3[:], w3t_s[:], segf_s[:], start=True, stop=True)
                r3 = small.tile([D, G], F32, name="r3", tag="r3")
                nc.scalar.activation(r3[:], p3[:], mybir.ActivationFunctionType.Relu,
                                     bias=b3_s[:])
                p4 = pf_pool.tile([NCLASS, G], F32, name="p4", tag="pf")
                nc.tensor.matmul(p4[:], w4t_s[:], r3[:], start=True, stop=True)
                l4 = small.tile([NCLASS, G], F32, name="l4", tag="l4")
                nc.scalar.activation(l4[:], p4[:],
                                     mybir.ActivationFunctionType.Identity,
                                     bias=b4_s[:])

                # ---- transpose logits to [G, NCLASS]; log_softmax over free ----
                pt = pf_pool.tile([G, NCLASS], F32, name="pt", tag="pf")
                nc.tensor.transpose(pt[:], l4[:], eye_s[0:NCLASS, 0:NCLASS])
                negmx = small.tile([G, 1], F32, name="negmx", tag="negmx")
                nc.vector.tensor_reduce(negmx[:], pt[:], axis=mybir.AxisListType.X,
                                        op=mybir.AluOpType.max, negate=True)
                ex = small.tile([G, NCLASS], F32, name="ex", tag="ex")
                nc.scalar.activation(ex[:], pt[:], mybir.ActivationFunctionType.Exp,
                                     bias=negmx[:])
                sm = small.tile([G, 1], F32, name="sm", tag="sm")
                nc.vector.reduce_sum(sm[:], ex[:], axis=mybir.AxisListType.X)
                ls = small.tile([G, 1], F32, name="ls", tag="ls")
                nc.scalar.activation(ls[:], sm[:], mybir.ActivationFunctionType.Ln)
                res = small.tile([G, NCLASS], F32, name="res", tag="res")
                nc.vector.tensor_scalar(res[:], pt[:], negmx[:], ls[:],
                                        op0=mybir.AluOpType.add,
                                        op1=mybir.AluOpType.subtract)
                nc.sync.dma_start(out[:], res[:])

    nc.compile()
    return nc


def _prep_inputs(inputs):
    """Host-side sharding/layout prep. Returns per-core input maps."""
    np_adj = _np_adj_dt()
    x_in = np.ascontiguousarray(np.asarray(inputs["x_in"], dtype=np.float32))
    adj = np.asarray(inputs["adj"], dtype=np.float32)
    idx = np.asarray(inputs["idx"]).astype(np.int64)
    W1 = np.asarray(inputs["W1"], dtype=np.float32)
    b1 = np.asarray(inputs["b1"], dtype=np.float32)
    W2 = np.asarray(inputs["W2"], dtype=np.float32)
    b2 = np.asarray(inputs["b2"], dtype=np.float32)
    W3 = np.asarray(inputs["W3"], dtype=np.float32)
    b3 = np.asarray(inputs["b3"], dtype=np.float32)
    W4 = np.asarray(inputs["W4"], dtype=np.float32)
    b4 = np.asarray(inputs["b4"], dtype=np.float32)

    # x in SBUF layout: xw[p, c*D + d] = x_in[c*128 + p, d]
    xw = np.ascontiguousarray(
        x_in.reshape(N // 128, 128, D).transpose(1, 0, 2).reshape(128, (N // 128) * D)
    ).astype(np_adj)

    w1t_aug = np.ascontiguousarray(np.concatenate([W1.T, b1[None, :]], axis=0))
    w2t_aug = np.ascontiguousarray(np.concatenate([W2.T, b2[None, :]], axis=0))
    w3t = np.ascontiguousarray(W3.T)
    w4t = np.ascontiguousarray(W4.T)
    b3c = np.ascontiguousarray(b3.reshape(D, 1))
    b4c = np.ascontiguousarray(b4.reshape(NCLASS, 1))
    eye = np.eye(32, dtype=np.float32)

    shared = {
        "xw": xw, "w1t": w1t_aug, "w2t": w2t_aug, "w3t": w3t,
        "b3": b3c, "w4t": w4t, "b4": b4c, "eye": eye,
    }

    in_maps = []
    for c in range(NCORES):
        r0 = c * ROWS
        at_c = np.ascontiguousarray(adj[r0:r0 + ROWS, :].T)  # [N, ROWS]
        at_c[r0 + np.arange(ROWS), np.arange(ROWS)] += 1.0   # fold in self-loops
        at_c = at_c.astype(np_adj, copy=False)

        # one-hot segment matrix in SBUF layout: st[p, b*G + g] =
        # 1 if idx[r0 + b*128 + p] == g
        st_c = np.zeros((128, IB * G), dtype=np.float32)
        loc = idx[r0:r0 + ROWS]
        p = np.arange(ROWS) % 128
        blk = np.arange(ROWS) // 128
        st_c[p, blk * G + loc] = 1.0

        in_maps.append({"at": at_c, "st": st_c, **shared})
    return in_maps


def run(inputs, trace=False):
    """Build (cached), shard, execute on 8 cores; returns (out, results)."""
    adj_dt = _adj_dt()
    key = ADJ_DT_NAME
    if key not in _BUILD_CACHE:
        _BUILD_CACHE[key] = _build(adj_dt)
    nc = _BUILD_CACHE[key]
    in_maps = _prep_inputs(inputs)
    res = run_bass_kernel_spmd(nc, in_maps, core_ids=list(range(NCORES)),
                               trace=trace)
    return np.asarray(res.results[0]["out"], dtype=np.float32), res


def kernel(**inputs):
    out, _ = run(inputs, trace=False)
    return out


# revision 6
# speedup vs baseline: 2.5812x; 2.5812x over previous
"""Trainium2 Bass kernel for a 2-layer GNN (dense message passing) + MLP readout.

Reference computation (N=16384 nodes, D=64 features, G=128 graphs):
    adj_hat = adj + I
    x  = adj_hat @ x_in                 # prop 1
    x  = relu(x @ W1.T + b1)            # fc1
    x  = adj_hat @ x                    # prop 2
    x  = relu(x @ W2.T + b2)            # fc2
    out = segment_sum(x, idx, 128)      # readout
    out = relu(out @ W3.T + b3)
    out = out @ W4.T + b4
    return log_softmax(out, axis=1)

Sharding over 8 NeuronCores: row-shard adj_hat over output nodes (2048 rows
per core). Each core receives its shard pre-transposed (A_T[j, i_local], with
self-loops folded in on the host) so the contraction index j lands on SBUF
partitions. The propagation matmul streams A_T tiles as the moving operand
with x chunks as the stationary operand, producing h.T in PSUM. The small fc
layers run with the bias folded in via an appended ones-row. h1 is
all-gathered between the propagation steps; the segment-sum readout uses a
host-built one-hot matrix as a matmul and an all-reduce of the [64, 128]
partial, after which every core computes the tiny readout MLP + log_softmax.
"""

import os
import sys

for _p in ("/opt/trn_rl_repo",):
    if _p not in sys.path and os.path.isdir(_p):
        sys.path.insert(0, _p)

import numpy as np

import concourse.bass as bass
import concourse.bacc as bacc
import concourse.tile as tile
import concourse.mybir as mybir
from concourse.bass_utils import run_bass_kernel_spmd

F32 = mybir.dt.float32

N = 16384          # nodes
D = 64             # feature dim (== H1 == H2 == H3)
G = 128            # graphs
NCLASS = 10
NCORES = 8
ROWS = N // NCORES          # 2048 output rows per core
JCHUNK = 128                # contraction tile (partition dim)
NJC = N // JCHUNK           # 128 j-chunks
NBANK = 4                   # psum accumulator banks per propagation
BCOLS = ROWS // NBANK       # 512 columns per bank
IB = ROWS // 128            # 16 output-row blocks of 128

# dtype of the adjacency streaming path (adj shard, x/h1 matmul weights).
# "float32r" is bit-identical to float32 in memory but streams through the
# PE at full rate (fp32 runs at 1/4 rate).
ADJ_DT_NAME = "float16"

_BUILD_CACHE = {}


def _adj_dt():
    return getattr(mybir.dt, ADJ_DT_NAME)


def _np_adj_dt():
    return mybir.dt.np(_adj_dt())


def _build(adj_dt, reps=1):
    nc = bacc.Bacc("TRN2", target_bir_lowering=False, debug=False,
                   enable_asserts=True, num_devices=NCORES)

    at = nc.dram_tensor("at", [N, ROWS], adj_dt, kind="ExternalInput").ap()
    xw = nc.dram_tensor("xw", [128, (N // 128) * D], adj_dt, kind="ExternalInput").ap()
    st = nc.dram_tensor("st", [128, IB * G], F32, kind="ExternalInput").ap()
    w1t = nc.dram_tensor("w1t", [D + 1, D], F32, kind="ExternalInput").ap()
    w2t = nc.dram_tensor("w2t", [D + 1, D], F32, kind="ExternalInput").ap()
    w3t = nc.dram_tensor("w3t", [D, D], F32, kind="ExternalInput").ap()
    b3 = nc.dram_tensor("b3", [D, 1], F32, kind="ExternalInput").ap()
    w4t = nc.dram_tensor("w4t", [D, NCLASS], F32, kind="ExternalInput").ap()
    b4 = nc.dram_tensor("b4", [NCLASS, 1], F32, kind="ExternalInput").ap()
    eye = nc.dram_tensor("eye", [32, 32], F32, kind="ExternalInput").ap()
    out = nc.dram_tensor("out", [G, NCLASS], F32, kind="ExternalOutput").ap()

    groups = [list(range(NCORES))]

    with tile.TileContext(nc) as tc:
        with (
            tc.tile_pool(name="const", bufs=1) as const,
            tc.tile_pool(name="astream", bufs=8) as astream,
            tc.tile_pool(name="wstream", bufs=4) as wstream,
            tc.tile_pool(name="hp_pool", bufs=2) as hp_pool,
            tc.tile_pool(name="hb_pool", bufs=4) as hb_pool,
            tc.tile_pool(name="small", bufs=1) as small,
            tc.tile_pool(name="acc_pool", bufs=4, space="PSUM") as acc_pool,
            tc.tile_pool(name="pf_pool", bufs=2, space="PSUM") as pf_pool,
            tc.tile_pool(name="seg_pool", bufs=1, space="PSUM") as seg_pool,
            tc.tile_pool(name="dram", bufs=1, space="DRAM") as dram,
        ):
            # ---- constants into SBUF ----
            x_all = const.tile([128, (N // 128) * D], adj_dt)
            nc.sync.dma_start(x_all[:], xw[:])
            st_all = const.tile([128, IB * G], F32)
            nc.sync.dma_start(st_all[:], st[:])
            w1t_s = const.tile([D + 1, D], F32)
            nc.sync.dma_start(w1t_s[:], w1t[:])
            w2t_s = const.tile([D + 1, D], F32)
            nc.sync.dma_start(w2t_s[:], w2t[:])
            w3t_s = const.tile([D, D], F32)
            nc.sync.dma_start(w3t_s[:], w3t[:])
            b3_s = const.tile([D, 1], F32)
            nc.sync.dma_start(b3_s[:], b3[:])
            w4t_s = const.tile([D, NCLASS], F32)
            nc.sync.dma_start(w4t_s[:], w4t[:])
            b4_s = const.tile([NCLASS, 1], F32)
            nc.sync.dma_start(b4_s[:], b4[:])
            eye_s = const.tile([32, 32], F32)
            nc.sync.dma_start(eye_s[:], eye[:])

            def propagate(weight_of_chunk):
                """One propagation: h.T = sum_j w[j,:].T * A_T[j,:] into PSUM,
                then copied to a [65, 2048] SBUF tile with a trailing ones row
                (for the bias-folding fc matmul). Returns the SBUF tile."""
                acc = [
                    acc_pool.tile([D, BCOLS], F32, name=f"acc{b}", tag="acc")
                    for b in range(NBANK)
                ]
                for jc in range(NJC):
                    a_t = astream.tile([128, ROWS], adj_dt, name="a_t", tag="a")
                    nc.sync.dma_start(a_t[:], at[jc * 128:(jc + 1) * 128, :])
                    wsl = weight_of_chunk(jc)
                    for b in range(NBANK):
                        nc.tensor.matmul(
                            acc[b][:],
                            wsl,
                            a_t[:, b * BCOLS:(b + 1) * BCOLS],
                            start=(jc == 0),
                            stop=(jc == NJC - 1),
                        )
                hp = hp_pool.tile([D + 1, ROWS], F32, name="hp", tag="hp")
                nc.vector.memset(hp[D:D + 1, :], 1.0)
                for b in range(NBANK):
                    nc.vector.tensor_copy(hp[0:D, b * BCOLS:(b + 1) * BCOLS], acc[b][:])
                return hp

            for _rep in range(reps):
                h1_loc = dram.tile([ROWS, D], adj_dt, name="h1_loc")
                h1_full = dram.tile([N, D], adj_dt, name="h1_full",
                                    addr_space="Shared")
                seg_loc = dram.tile([D, G], F32, name="seg_loc")
                seg_full = dram.tile([D, G], F32, name="seg_full",
                                     addr_space="Shared")

                # ---- propagation 1 (weights: x chunks already resident) ----
                hp1 = propagate(lambda jc: x_all[:, jc * D:(jc + 1) * D])

                # ---- fc1 (+bias via ones row) -> relu -> h1 natural -> DRAM ----
                for ib in range(IB):
                    pf = pf_pool.tile([128, D], F32, name="pf1", tag="pf")
                    nc.tensor.matmul(pf[:], hp1[:, ib * 128:(ib + 1) * 128], w1t_s[:],
                                     start=True, stop=True)
                    hb = hb_pool.tile([128, D], adj_dt, name="hb1", tag="hb1")
                    nc.scalar.activation(hb[:], pf[:],
                                         mybir.ActivationFunctionType.Relu)
                    nc.sync.dma_start(h1_loc[ib * 128:(ib + 1) * 128, :], hb[:])

                nc.gpsimd.collective_compute(
                    "AllGather", mybir.AluOpType.bypass, replica_groups=groups,
                    ins=[h1_loc.opt()], outs=[h1_full.opt()],
                )

                # ---- propagation 2 (weights: gathered h1 chunks) ----
                def h1_weight(jc):
                    w_t = wstream.tile([128, D], adj_dt, name="w_t", tag="w")
                    nc.sync.dma_start(w_t[:], h1_full[jc * 128:(jc + 1) * 128, :])
                    return w_t[:]

                hp2 = propagate(h1_weight)

                # ---- fc2 -> relu -> h2 natural; readout partial via one-hot ----
                seg_ps = seg_pool.tile([D, G], F32, name="seg_ps", tag="seg")
                for ib in range(IB):
                    pf = pf_pool.tile([128, D], F32, name="pf2", tag="pf")
                    nc.tensor.matmul(pf[:], hp2[:, ib * 128:(ib + 1) * 128], w2t_s[:],
                                     start=True, stop=True)
                    hb2 = hb_pool.tile([128, D], F32, name="hb2", tag="hb2")
                    nc.scalar.activation(hb2[:], pf[:],
                                         mybir.ActivationFunctionType.Relu)
                    nc.tensor.matmul(seg_ps[:], hb2[:], st_all[:, ib * G:(ib + 1) * G],
                                     start=(ib == 0), stop=(ib == IB - 1))

                seg_s = small.tile([D, G], F32, name="seg_s", tag="seg_s")
                nc.vector.tensor_copy(seg_s[:], seg_ps[:])
                nc.sync.dma_start(seg_loc[:], seg_s[:])
                nc.gpsimd.collective_compute(
                    "AllReduce", mybir.AluOpType.add, replica_groups=groups,
                    ins=[seg_loc.opt()], outs=[seg_full.opt()],
                )
                segf_s = small.tile([D, G], F32, name="segf_s", tag="segf_s")
                nc.sync.dma_start(segf_s[:], seg_full[:])

                # ---- readout MLP: fc3 relu, fc4 (+bias), all in .T layout ----
                p3 = pf_pool.tile([D, G], F32, name="p3", tag="pf")
                nc.tensor.matmul(p3[:], w3t_s[:], segf_s[:], start=True, stop=True)
                r3 = small.tile([D, G], F32, name="r3", tag="r3")
                nc.scalar.activation(r3[:], p3[:], mybir.ActivationFunctionType.Relu,
                                     bias=b3_s[:])
                p4 = pf_pool.tile([NCLASS, G], F32, name="p4", tag="pf")
                nc.tensor.matmul(p4[:], w4t_s[:], r3[:], start=True, stop=True)
                l4 = small.tile([NCLASS, G], F32, name="l4", tag="l4")
                nc.scalar.activation(l4[:], p4[:],
                                     mybir.ActivationFunctionType.Identity,
                                     bias=b4_s[:])

                # ---- transpose logits to [G, NCLASS]; log_softmax over free ----
                pt = pf_pool.tile([G, NCLASS], F32, name="pt", tag="pf")
                nc.tensor.transpose(pt[:], l4[:], eye_s[0:NCLASS, 0:NCLASS])
                negmx = small.tile([G, 1], F32, name="negmx", tag="negmx")
                nc.vector.tensor_reduce(negmx[:], pt[:], axis=mybir.AxisListType.X,
                                        op=mybir.AluOpType.max, negate=True)
                ex = small.tile([G, NCLASS], F32, name="ex", tag="ex")
                nc.scalar.activation(ex[:], pt[:], mybir.ActivationFunctionType.Exp,
                                     bias=negmx[:])
                sm = small.tile([G, 1], F32, name="sm", tag="sm")
                nc.vector.reduce_sum(sm[:], ex[:], axis=mybir.AxisListType.X)
                ls = small.tile([G, 1], F32, name="ls", tag="ls")
                nc.scalar.activation(ls[:], sm[:], mybir.ActivationFunctionType.Ln)
                res = small.tile([G, NCLASS], F32, name="res", tag="res")
                nc.vector.tensor_scalar(res[:], pt[:], negmx[:], ls[:],
                                        op0=mybir.AluOpType.add,
                                        op1=mybir.AluOpType.subtract)
                nc.sync.dma_start(out[:], res[:])

    nc.compile()
    return nc


def _prep_inputs(inputs):
    """Host-side sharding/layout prep. Returns per-core input maps."""
    np_adj = _np_adj_dt()
    x_in = np.ascontiguousarray(np.asarray(inputs["x_in"], dtype=np.float32))
    adj = np.asarray(inputs["adj"], dtype=np.float32)
    idx = np.asarray(inputs["idx"]).astype(np.int64)
    W1 = np.asarray(inputs["W1"], dtype=np.float32)
    b1 = np.asarray(inputs["b1"], dtype=np.float32)
    W2 = np.asarray(inputs["W2"], dtype=np.float32)
    b2 = np.asarray(inputs["b2"], dtype=np.float32)
    W3 = np.asarray(inputs["W3"], dtype=np.float32)
    b3 = np.asarray(inputs["b3"], dtype=np.float32)
    W4 = np.asarray(inputs["W4"], dtype=np.float32)
    b4 = np.asarray(inputs["b4"], dtype=np.float32)

    # x in SBUF layout: xw[p, c*D + d] = x_in[c*128 + p, d]
    xw = np.ascontiguousarray(
        x_in.reshape(N // 128, 128, D).transpose(1, 0, 2).reshape(128, (N // 128) * D)
    ).astype(np_adj)

    w1t_aug = np.ascontiguousarray(np.concatenate([W1.T, b1[None, :]], axis=0))
    w2t_aug = np.ascontiguousarray(np.concatenate([W2.T, b2[None, :]], axis=0))
    w3t = np.ascontiguousarray(W3.T)
    w4t = np.ascontiguousarray(W4.T)
    b3c = np.ascontiguousarray(b3.reshape(D, 1))
    b4c = np.ascontiguousarray(b4.reshape(NCLASS, 1))
    eye = np.eye(32, dtype=np.float32)

    shared = {
        "xw": xw, "w1t": w1t_aug, "w2t": w2t_aug, "w3t": w3t,
        "b3": b3c, "w4t": w4t, "b4": b4c, "eye": eye,
    }

    in_maps = []
    for c in range(NCORES):
        r0 = c * ROWS
        at_c = np.ascontiguousarray(adj[r0:r0 + ROWS, :].T)  # [N, ROWS]
        at_c[r0 + np.arange(ROWS), np.arange(ROWS)] += 1.0   # fold in self-loops
        at_c = at_c.astype(np_adj, copy=False)

        # one-hot segment matrix in SBUF layout: st[p, b*G + g] =
        # 1 if idx[r0 + b*128 + p] == g
        st_c = np.zeros((128, IB * G), dtype=np.float32)
        loc = idx[r0:r0 + ROWS]
        p = np.arange(ROWS) % 128
        blk = np.arange(ROWS) // 128
        st_c[p, blk * G + loc] = 1.0

        in_maps.append({"at": at_c, "st": st_c, **shared})
    return in_maps


def run(inputs, trace=False):
    """Build (cached), shard, execute on 8 cores; returns (out, results)."""
    adj_dt = _adj_dt()
    key = ADJ_DT_NAME
    if key not in _BUILD_CACHE:
        _BUILD_CACHE[key] = _build(adj_dt)
    nc = _BUILD_CACHE[key]
    in_maps = _prep_inputs(inputs)
    res = run_bass_kernel_spmd(nc, in_maps, core_ids=list(range(NCORES)),
                               trace=trace)
    return np.asarray(res.results[0]["out"], dtype=np.float32), res


def kernel(**inputs):
    out, _ = run(inputs, trace=False)
    return out
